# revision 3
# baseline (speedup 1.0000x reference)
"""Multi-head causal attention (B=4, T=2048, C=1024, H=16, D=64) on 8 TRN2
NeuronCores via Bass/Tile — v2.

Sharding (as v1): core = 2*batch + g; g selects 8 of 16 heads. Each core
computes QKV projections (f32r matmuls), causal attention for its 8 heads
(scores f32r, P/V bf16, post-exp causal mask via gpsimd affine_select on the
idle Pool engine, softmax denominator via ones-augmented V column), then the
pair AllGathers the normalized attention outputs (bf16, 4x less wire than
v1's ReduceScatter of out-proj partials) and each core runs the output
projection for its own 512 output channels (w_out column split, bf16).

v2 speedups vs v1: all matmuls 1 cycle/row (f32r/bf16 vs fp32's 4), exp
activations grouped into [128,1024] two-bank PSUM spans (2x fewer, wider),
mask moved off the DVE critical path, smaller collective payloads, out-proj
software-pipelined one chunk behind the gather.
"""
import sys

for _p in ("/opt/trn_rl_repo", "/root/.axon_site/_ro/trn_rl_repo"):
    if _p not in sys.path:
        sys.path.append(_p)

import numpy as np
import concourse.bass as bass
import concourse.tile as tile
from concourse import bacc, mybir
from concourse.bass_utils import run_bass_kernel_spmd

F32 = mybir.dt.float32
F32R = mybir.dt.float32r
BF16 = mybir.dt.bfloat16

B, T, C = 4, 2048, 1024
H, D = 16, 64
NCORES = 8
HL = 8            # local heads per core
CL = HL * D       # 512 local channels
TCH = 512         # t-chunk (query tile)
NJ = T // TCH     # 4 chunks
KT = C // 128     # 8 contraction tiles for projections
KO = CL // 128    # 4 x 128-row groups of local channels


def build(collective=True, reps=1, num_devices=NCORES):
    nc = bacc.Bacc("TRN2", target_bir_lowering=False, debug=False,
                   num_devices=num_devices)
    x_t = nc.dram_tensor("x_t", [C, T], F32R, kind="ExternalInput").ap()
    w_qkv = nc.dram_tensor("w_qkv", [C, 3 * CL], F32R, kind="ExternalInput").ap()
    w_out = nc.dram_tensor("w_out", [C, CL], F32, kind="ExternalInput").ap()
    b_eff = nc.dram_tensor("b_eff", [CL], F32, kind="ExternalInput").ap()
    out_half = nc.dram_tensor("out_half", [CL, T], F32, kind="ExternalOutput").ap()

    with tile.TileContext(nc) as tc:
        with (
            tc.tile_pool(name="consts", bufs=1) as consts,
            tc.tile_pool(name="weights", bufs=1) as weights,
            tc.tile_pool(name="kv", bufs=1) as kv,
            tc.tile_pool(name="xin", bufs=2) as xin,
            tc.tile_pool(name="qp", bufs=2) as qp,
            tc.tile_pool(name="pp", bufs=3) as pp,
            tc.tile_pool(name="att", bufs=2) as att,
            tc.tile_pool(name="sm", bufs=2) as sm,
            tc.tile_pool(name="agp", bufs=3) as agp,
            tc.tile_pool(name="outp", bufs=2) as outp,
            tc.tile_pool(name="ps_proj", bufs=2, space="PSUM") as ps_proj,
            tc.tile_pool(name="ps_s", bufs=2, space="PSUM") as ps_s,
            tc.tile_pool(name="ps_pv", bufs=2, space="PSUM") as ps_pv,
            tc.tile_pool(name="dram", bufs=4, space="DRAM") as dram,
        ):
            # causal boundary mask: keep 0 where col >= partition, else NEG
            mask = consts.tile([128, 128], F32)
            nc.vector.memset(mask[:], 0.0)
            nc.gpsimd.affine_select(
                out=mask[:], in_=mask[:], compare_op=mybir.AluOpType.is_ge,
                fill=-1e10, base=0, pattern=[[1, 128]], channel_multiplier=-1)
            b_sb = consts.tile([128, KO], F32)
            nc.sync.dma_start(b_sb[:], b_eff.rearrange("(mo p) -> p mo", p=128))

            # ---- weights ----
            w_t = weights.tile([128, KT, 3 * CL], F32R)
            w_r = w_qkv.rearrange("(kt p) n -> p kt n", p=128)
            for k in range(KT):
                nc.sync.dma_start(w_t[:, k, :], w_r[:, k, :])
            wo_t = weights.tile([128, KT, CL], BF16)
            nc.gpsimd.dma_start(wo_t[:], w_out.rearrange("(kt p) n -> p kt n", p=128))

            # ---- persistent K^T and (ones-augmented) V ----
            kt_t = kv.tile([128, KO, T], F32R)       # rows = local qk-chan, cols = t
            v_t = kv.tile([128, T // 128, HL * 65], BF16)  # rows = t, 65 cols/head
            v_aug = v_t.rearrange("p tt (h e) -> p tt h e", e=65)
            nc.vector.memset(v_aug[:, :, :, 64:65], 1.0)

            x_r = x_t.rearrange("(kt p) t -> p kt t", p=128)
            wo_r = wo_t.rearrange("p kt (mo f) -> p kt mo f", f=128)

            def emit_outproj(jj, ag):
                ts = slice(jj * TCH, (jj + 1) * TCH)
                for mo in range(KO):
                    pso = ps_proj.tile([128, TCH], F32, tag="proj")
                    for kb in range(KT):
                        nc.tensor.matmul(
                            pso[:], wo_r[:, kb, mo, :], ag[:, kb, :],
                            start=(kb == 0), stop=(kb == KT - 1))
                    ob = outp.tile([128, TCH], F32)
                    nc.vector.tensor_scalar_add(ob[:], pso[:], b_sb[:, mo:mo + 1])
                    nc.sync.dma_start(out_half[128 * mo:128 * (mo + 1), ts], ob[:])

            pending = []
            for _rep in range(reps):
              for j in range(NJ):
                ts = slice(j * TCH, (j + 1) * TCH)
                # ---- load x^T chunk ----
                xc = xin.tile([128, KT, TCH], F32R)
                nc.sync.dma_start(xc[:], x_r[:, :, ts])

                # ---- projections for this chunk (all f32r, 1 cycle/row) ----
                qt_c = qp.tile([128, KO, TCH], F32R)
                for m in range(KO):
                    psq = ps_proj.tile([128, TCH], F32, tag="proj")
                    for k in range(KT):
                        nc.tensor.matmul(
                            psq[:], w_t[:, k, 128 * m:128 * (m + 1)], xc[:, k, :],
                            start=(k == 0), stop=(k == KT - 1))
                    nc.vector.tensor_copy(qt_c[:, m, :], psq[:])
                for m in range(KO):
                    psk = ps_proj.tile([128, TCH], F32, tag="proj")
                    for k in range(KT):
                        nc.tensor.matmul(
                            psk[:], w_t[:, k, CL + 128 * m:CL + 128 * (m + 1)],
                            xc[:, k, :], start=(k == 0), stop=(k == KT - 1))
                    nc.vector.tensor_copy(kt_t[:, m, ts], psk[:])
                for ttl in range(TCH // 128):
                    tt = j * (TCH // 128) + ttl
                    psv = ps_proj.tile([128, CL], F32, tag="proj")
                    for k in range(KT):
                        nc.tensor.matmul(
                            psv[:], xc[:, k, 128 * ttl:128 * (ttl + 1)],
                            w_t[:, k, 2 * CL:3 * CL],
                            start=(k == 0), stop=(k == KT - 1))
                    nc.vector.tensor_copy(
                        v_aug[:, tt, :, 0:64],
                        psv.rearrange("p (h d) -> p h d", h=HL))

                # ---- attention for this chunk ----
                at_c = att.tile([128, KO, TCH], BF16)
                for m in range(KO):
                    ha, hb = 2 * m, 2 * m + 1
                    pva = ps_pv.tile([65, TCH], F32, tag="pv")
                    pvb = ps_pv.tile([65, TCH], F32, tag="pv")
                    nkb = 4 * (j + 1)
                    for kb in range(nkb):
                        r = kb - 4 * j
                        off = 128 * max(r, 0)
                        ks = slice(128 * kb, 128 * (kb + 1))
                        # scores^T for both heads into one 2-bank PSUM tile,
                        # masked cols (q < k) never computed
                        sp = ps_s.tile([128, 2 * TCH], F32, tag="s")
                        if off:
                            # zero the never-computed gap so one exp can span
                            # the whole slot (gap output is never read)
                            nc.vector.memset(sp[:, TCH:TCH + off], 0.0)
                        nc.tensor.matmul(
                            sp[:, off:TCH], kt_t[0:64, m, ks], qt_c[0:64, m, off:])
                        nc.tensor.matmul(
                            sp[:, TCH + off:], kt_t[64:128, m, ks],
                            qt_c[64:128, m, off:])
                        if r >= 0:
                            # boundary triangle: -inf where q < k inside block
                            nc.vector.tensor_add(
                                sp[:, off:off + 128], sp[:, off:off + 128], mask[:])
                            nc.vector.tensor_add(
                                sp[:, TCH + off:TCH + off + 128],
                                sp[:, TCH + off:TCH + off + 128], mask[:])
                        # exp -> bf16 P: one act per kb spanning both banks;
                        # for diagonal blocks the [TCH:TCH+off] gap region is
                        # garbage-but-finite and never read downstream
                        p2 = pp.tile([128, 2 * TCH], BF16)
                        nc.scalar.activation(
                            p2[:, off:], sp[:, off:],
                            mybir.ActivationFunctionType.Exp)
                        nc.tensor.matmul(
                            pva[:, off:], v_t[:, kb, 65 * ha:65 * ha + 65],
                            p2[:, off:TCH], start=(kb == 0), stop=(kb == nkb - 1))
                        nc.tensor.matmul(
                            pvb[:, off:], v_t[:, kb, 65 * hb:65 * hb + 65],
                            p2[:, TCH + off:], start=(kb == 0), stop=(kb == nkb - 1))
                    for half, pv in ((0, pva), (1, pvb)):
                        r0 = 64 * half
                        # copy out of PSUM first so the bank frees fast
                        pc = sm.tile([65, TCH], F32, tag="pc")
                        nc.vector.tensor_copy(pc[:], pv[:])
                        rc = sm.tile([1, TCH], F32, tag="rc")
                        nc.vector.reciprocal(rc[:], pc[64:65, :])
                        bc = sm.tile([64, TCH], F32, tag="bc")
                        nc.gpsimd.partition_broadcast(bc[:], rc[:])
                        nc.vector.tensor_mul(at_c[r0:r0 + 64, m, :], pc[0:64, :], bc[:])

                # ---- pair AllGather of normalized attention output (bf16) ----
                cc_in = dram.tile([CL, TCH], BF16)
                nc.sync.dma_start(
                    cc_in.rearrange("(ko p) q -> p ko q", p=128), at_c[:])
                if collective:
                    cc_out = dram.tile([2 * CL, TCH], BF16, tag="cc2")
                    nc.gpsimd.collective_compute(
                        "AllGather", mybir.AluOpType.bypass,
                        replica_groups=[[0, 1], [2, 3], [4, 5], [6, 7]],
                        ins=[cc_in.opt()], outs=[cc_out.opt()])
                    ag_src = cc_out
                else:
                    cc2 = dram.tile([2 * CL, TCH], BF16, tag="cc2")
                    nc.sync.dma_start(cc2[0:CL, :], cc_in[:])
                    nc.sync.dma_start(cc2[CL:, :], cc_in[:])
                    ag_src = cc2
                ag = agp.tile([128, KT, TCH], BF16)
                nc.sync.dma_start(
                    ag[:], ag_src.rearrange("(kt p) q -> p kt q", p=128))

                pending.append((j, ag))
                if len(pending) > 2:
                    emit_outproj(*pending.pop(0))
            while pending:
                emit_outproj(*pending.pop(0))

    nc.compile()
    return nc


_NC_CACHE = {}


def get_nc(collective=True, reps=1):
    key = (collective, reps)
    if key not in _NC_CACHE:
        _NC_CACHE[key] = build(collective, reps)
    return _NC_CACHE[key]


def make_in_maps(x, w_qkv, w_out, b_out):
    x = np.asarray(x, dtype=np.float32)
    w_qkv = np.asarray(w_qkv, dtype=np.float32)
    w_out = np.asarray(w_out, dtype=np.float32)
    b_out = np.asarray(b_out, dtype=np.float32)
    scale = np.float32(D ** -0.5)
    in_maps = []
    for c in range(NCORES):
        bi, g = c // 2, c % 2
        cols = slice(CL * g, CL * (g + 1))
        w_loc = np.concatenate(
            [w_qkv[:, cols] * scale, w_qkv[:, C:][:, cols], w_qkv[:, 2 * C:][:, cols]],
            axis=1)
        in_maps.append({
            "x_t": np.ascontiguousarray(x[bi].T),
            "w_qkv": np.ascontiguousarray(w_loc),
            "w_out": np.ascontiguousarray(w_out[:, cols]),
            "b_eff": np.ascontiguousarray(b_out[cols]),
        })
    return in_maps


def assemble(results):
    out = np.empty((B, T, C), dtype=np.float32)
    for bi in range(B):
        top = results[2 * bi]["out_half"]       # channels 0:512
        bot = results[2 * bi + 1]["out_half"]   # channels 512:1024
        out[bi] = np.concatenate([top, bot], axis=0).T
    return out


def kernel(x, w_qkv, w_out, b_out):
    nc = get_nc(collective=True)
    in_maps = make_in_maps(x, w_qkv, w_out, b_out)
    res = run_bass_kernel_spmd(nc, in_maps, list(range(NCORES)))
    return assemble(res.results)


if __name__ == "__main__":
    nc = build()
    print("instructions:", len(nc.inst_map))
